# revision 1
# baseline (speedup 1.0000x reference)
"""Trainium2 Bass kernel for GAT attention mechanism.

Reference computation (N=1024, F=F'=128):
    Vp    = V @ W.T + b                       # [N, F']
    score = s_i[:,None] + s_j[None,:] + einsum('ijf,f->ij', time_enc, a_t)
    score = leaky_relu(score, 0.01)
    att   = softmax(score, axis=-1)
    H     = att[:,:,None] * Vp[None,:,:] * E[:,:,None]   # [N, N, F']

Sharding: row-blocks of i across 8 cores (128 rows each). Each core gets its
slice of time_enc/E plus replicated V/params; softmax over j stays local.

Implementation notes:
  * time_enc and H are carried in bf16 (host-side cast); this halves the
    dominant HBM traffic (64+64 MB -> 32+32 MB per core). Scores accumulate
    in fp32 PSUM, so the only precision loss is bf16 rounding of the inputs
    to the te.a_t dot product and of the final H elements (rel err ~4e-3,
    gate is 2e-2).
  * Phase A (scores) runs on the tensor engine: te arrives transposed via
    DMA-transpose as teT[f, j] per row i, then one matvec per (i, j-half)
    accumulates score row i into PSUM. s_i/s_j enter as two PSUM-accumulated
    rank-F matmuls, so the full pre-activation score materializes in PSUM.
  * Phase B (H = w * Vp) is elementwise tensor_scalar work split DVE/ACT,
    with j interleaved as j = p*nq+q so stores are 2KB-contiguous per
    partition. Stores go out on the gpsimd (SWDGE) ring, keeping both
    HWDGE rings free for the next iteration's te loads.
"""

import sys

sys.path.insert(0, "/opt/trn_rl_repo")

import numpy as np
import ml_dtypes

import concourse.bass as bass
import concourse.tile as tile
from concourse import bacc, mybir
from concourse.masks import make_identity

F32 = mybir.dt.float32
BF16 = mybir.dt.float16
NP_BF16 = np.float16

N = 1024          # nodes (j dimension, full)
F = 128           # feature dim (= F_OUT = F_IN)
M = 8             # cores
NL = N // M       # local i rows per core (128)
GI = 8            # i rows per transposed te load (2 MB per DMA)
IB = 8            # i rows per H store DMA (2 MB)
NQ = N // 128     # j interleave factor (8)
NT = N // 128     # V row tiles
NEG_SLOPE = 0.01
ACT_Q = 2         # of the NQ j-blocks in phase B, this many go to ACT


def build_kernel(n=N, nl=NL, reps=1, debug=False):
    nq = NQ
    nt = NT

    nc = bacc.Bacc()
    if debug:
        teT_dbg = nc.dram_tensor("teT_dbg", [F, GI, n], BF16, kind="ExternalOutput")
        score_dbg = nc.dram_tensor("score_dbg", [nl, n], F32, kind="ExternalOutput")
        w_dbg = nc.dram_tensor("w_dbg", [nl, n], F32, kind="ExternalOutput")
        vpb_dbg = nc.dram_tensor("vpb_dbg", [128, NQ, F], BF16, kind="ExternalOutput")
    te_d = nc.dram_tensor("te", [nl, n, F], BF16, kind="ExternalInput")
    E_d = nc.dram_tensor("E", [nl, n], F32, kind="ExternalInput")
    V_d = nc.dram_tensor("V", [n, F], F32, kind="ExternalInput")
    Vloc_d = nc.dram_tensor("Vloc", [nl, F], F32, kind="ExternalInput")
    W_d = nc.dram_tensor("W", [F, F], F32, kind="ExternalInput")
    b_d = nc.dram_tensor("b", [F, 1], F32, kind="ExternalInput")
    a_d = nc.dram_tensor("a", [3 * F, 1], F32, kind="ExternalInput")
    atb_d = nc.dram_tensor("atb", [F, 1], BF16, kind="ExternalInput")
    ajr_d = nc.dram_tensor("ajr", [F, 128], BF16, kind="ExternalInput")
    H_d = nc.dram_tensor("H", [nl, n, F], BF16, kind="ExternalOutput")

    mult = mybir.AluOpType.mult
    add = mybir.AluOpType.add
    amax = mybir.AluOpType.max
    amin = mybir.AluOpType.min
    ident_fn = mybir.ActivationFunctionType.Identity

    with tile.TileContext(nc) as tc:
        from contextlib import nullcontext
        with (
            tc.tile_pool(name="const", bufs=2) as cp,
            tc.tile_pool(name="psum", bufs=2, space="PSUM") as pp,
            tc.tile_pool(name="psA", bufs=2, space="PSUM") as ppA,
            tc.tile_pool(name="te", bufs=3) as tep,
            tc.tile_pool(name="soft", bufs=2) as sp,
            tc.tile_pool(name="hout", bufs=2) as hp,
            tc.For_i(0, reps, 1) if reps > 1 else nullcontext(),
        ):
            # ---- early small-const DMAs (front of HWDGE FIFOs) -------------
            b_col = cp.tile([F, 1], F32, tag="b")
            nc.scalar.dma_start(b_col, b_d[:, :])
            a_i_col = cp.tile([F, 1], F32, tag="ai")
            nc.scalar.dma_start(a_i_col, a_d[0:F, :])
            at_bf = cp.tile([F, 1], BF16, tag="at")
            nc.scalar.dma_start(at_bf, atb_d[:, :])
            aj_rep = cp.tile([F, 128], BF16, tag="ajr")
            nc.scalar.dma_start(aj_rep, ajr_d[:, :])
            W_sb = cp.tile([F, F], F32, tag="W")
            nc.scalar.dma_start(W_sb, W_d[:, :])
            V_sb = cp.tile([128, nt, F], F32, tag="V")
            nc.scalar.dma_start(V_sb, V_d[:, :].rearrange("(t p) f -> p t f", p=128))
            Vloc_sb = cp.tile([nl, F], F32, tag="Vl")
            nc.scalar.dma_start(Vloc_sb, Vloc_d[:, :])
            E_sb = cp.tile([nl, n], F32, tag="E")
            nc.scalar.dma_start(E_sb, E_d[:, :])

            ident = cp.tile([128, 128], F32, tag="id")
            make_identity(nc, ident)
            ident_bf = cp.tile([128, 128], BF16, tag="idb")
            make_identity(nc, ident_bf)
            ones_bf = cp.tile([F, 512], BF16, tag="ones")
            nc.vector.memset(ones_bf, 1.0)

            # ---- te transposed loads: first tiles on the FIFO early --------
            ngrp = nl // GI
            te_tiles = {}
            for t in range(min(2, ngrp)):
                te_t = tep.tile([F, GI, n], BF16, tag="te")
                eng = nc.sync  # transposes must all share one ring (xbar hazard)
                eng.dma_start_transpose(
                    te_t[:, :, :].rearrange("f g j -> f (g j)"),
                    te_d[t * GI : (t + 1) * GI].rearrange("g j f -> (g j) f"),
                )
                te_tiles[t] = te_t

            # ---- projection: VpT = W @ V.T + b (fp32), then bf16 copies ----
            Wt_sb = cp.tile([F, F], F32, tag="Wt")
            ps = pp.tile([128, 512], F32, tag="tp")
            nc.tensor.transpose(ps[:, :128], W_sb, ident)
            nc.scalar.copy(Wt_sb, ps[:, :128])

            VT_sb = cp.tile([F, nt, 128], F32, tag="VT")
            for t in range(nt):
                ps = pp.tile([128, 512], F32, tag="tp")
                nc.tensor.transpose(ps[:, :128], V_sb[:, t, :], ident)
                nc.scalar.copy(VT_sb[:, t, :], ps[:, :128])

            VpT_sb = cp.tile([F, nt, 128], F32, tag="VpT")
            for h in range(0, nt, 4):
                psw = pp.tile([128, 512], F32, tag="tp")
                nc.tensor.matmul(
                    psw,
                    Wt_sb,
                    VT_sb[:, h : h + 4, :].rearrange("p a b -> p (a b)"),
                )
                nc.scalar.activation(
                    VpT_sb[:, h : h + 4, :].rearrange("p a b -> p (a b)"),
                    psw,
                    ident_fn,
                    bias=b_col, scale=1.0,
                )
            VpT_bf = cp.tile([F, n], BF16, tag="VpTb")
            nc.vector.tensor_copy(VpT_bf, VpT_sb[:, :, :].rearrange("p a b -> p (a b)"))

            # Vp_blk[p, q, f] = Vp[p*nq+q, f] (bf16) for phase B
            Vp_blk = cp.tile([128, nq, F], BF16, tag="Vpb")
            for q in range(nq):
                psb = pp.tile([128, 512], BF16, tag="tpb")
                nc.tensor.transpose(psb[:, :128], VpT_bf[:, q::nq], ident_bf)
                nc.scalar.copy(Vp_blk[:, q, :], psb[:, :128])

            # local rows: VpTloc (fp32) -> VpAi_bf[f', i] = VpTloc * a_i
            VlocT_sb = cp.tile([F, nl], F32, tag="VlT")
            ps = pp.tile([128, 512], F32, tag="tp")
            nc.tensor.transpose(ps[:, :nl], Vloc_sb, ident)
            nc.scalar.copy(VlocT_sb, ps[:, :nl])
            VpTloc_sb = cp.tile([F, nl], F32, tag="VpTl")
            ps = pp.tile([128, 512], F32, tag="tp")
            nc.tensor.matmul(ps[:, :nl], Wt_sb, VlocT_sb)
            nc.scalar.activation(
                VpTloc_sb, ps[:, :nl], ident_fn, bias=b_col, scale=1.0,
            )
            VpAi_bf = cp.tile([F, nl], BF16, tag="VpAi")
            nc.vector.tensor_scalar_mul(VpAi_bf, VpTloc_sb, a_i_col)

            # sliding-window stationary: zeros except col 128 = a_t, so
            # A_win[:, 128-i:256-i] places a_t at stationary column i and
            # the matvec writes (only) score row i of the PSUM tile.
            A_win = cp.tile([F, 256], BF16, tag="Awin")
            nc.vector.memset(A_win, 0.0)
            nc.vector.tensor_copy(A_win[:, 128:129], at_bf)

            # ---- phase A: score rows accumulate in PSUM --------------------
            # ps_h[i, jh] = s_i[i] + s_j[jh] + sum_f a_t[f] teT_i[f, jh]
            psc = []
            for h in range(2):
                psc_h = ppA.tile([128, 512], F32, tag=f"sc{h}", name=f"psc{h}")
                psc.append(psc_h)
            for h in range(2):
                nc.tensor.matmul(
                    psc[h], aj_rep, VpT_bf[:, h * 512 : (h + 1) * 512],
                    start=True, stop=False,
                )
                nc.tensor.matmul(
                    psc[h], VpAi_bf, ones_bf, start=False, stop=False,
                )
            for t in range(ngrp):
                if t in te_tiles:
                    te_t = te_tiles[t]
                else:
                    te_t = tep.tile([F, GI, n], BF16, tag="te")
                    eng = nc.sync  # transposes must all share one ring (xbar hazard)
                    eng.dma_start_transpose(
                        te_t[:, :, :].rearrange("f g j -> f (g j)"),
                        te_d[t * GI : (t + 1) * GI].rearrange("g j f -> (g j) f"),
                    )
                if debug and t == 0:
                    nc.scalar.dma_start(teT_dbg[:, :, :], te_t)
                for g in range(GI):
                    i = t * GI + g
                    last = i == nl - 1
                    for h in range(2):
                        nc.tensor.matmul(
                            psc[h],
                            A_win[:, 128 - i : 256 - i],
                            te_t[:, g, h * 512 : (h + 1) * 512],
                            start=False, stop=last,
                        )

            # ---- softmax tail ----------------------------------------------
            score_pre = sp.tile([nl, n], F32, tag="spre")
            score2 = sp.tile([nl, n], F32, tag="s2")
            pmax = sp.tile([nl, 2], F32, tag="pmax")
            for h in range(2):
                sl = slice(h * 512, (h + 1) * 512)
                nc.scalar.activation(score_pre[:, sl], psc[h], ident_fn)
                nc.vector.tensor_reduce(
                    pmax[:, h : h + 1], psc[h],
                    axis=mybir.AxisListType.X, op=amax, negate=True,
                )
            neg_max_pre = sp.tile([nl, 1], F32, tag="nm")
            nc.vector.tensor_reduce(
                neg_max_pre, pmax, axis=mybir.AxisListType.X, op=amin,
            )
            # -leaky(max_pre) = min(neg_max_pre, 0.01*neg_max_pre)
            neg_m = sp.tile([nl, 1], F32, tag="nml")
            nc.vector.scalar_tensor_tensor(
                out=neg_m, in0=neg_max_pre, scalar=NEG_SLOPE,
                in1=neg_max_pre, op0=mult, op1=amin,
            )
            for h in range(2):
                sl = slice(h * 512, (h + 1) * 512)
                nc.vector.scalar_tensor_tensor(
                    out=score2[:, sl], in0=score_pre[:, sl], scalar=NEG_SLOPE,
                    in1=score_pre[:, sl], op0=mult, op1=amax,
                )
            if debug:
                nc.scalar.dma_start(score_dbg[:, :], score_pre)
            exps = sp.tile([nl, n], F32, tag="exps")
            row_sum = sp.tile([nl, 1], F32, tag="rs")
            nc.scalar.activation(
                exps, score2, mybir.ActivationFunctionType.Exp,
                bias=neg_m, scale=1.0, accum_out=row_sum,
            )
            rinv = sp.tile([nl, 1], F32, tag="ri")
            nc.vector.reciprocal(rinv, row_sum)
            w_sb = sp.tile([nl, n], F32, tag="w")
            for h in range(2):
                sl = slice(h * 512, (h + 1) * 512)
                nc.vector.scalar_tensor_tensor(
                    out=w_sb[:, sl], in0=exps[:, sl], scalar=rinv,
                    in1=E_sb[:, sl], op0=mult, op1=mult,
                )

            # ---- phase B: H[i, j, f] = w[i, j] * Vp[j, f], j = p*nq+q ------
            if debug:
                nc.scalar.dma_start(w_dbg[:, :], w_sb)
                nc.scalar.dma_start(vpb_dbg[:, :, :], Vp_blk)
            wT_blk = sp.tile([128, nq, nl], F32, tag="wT")
            for q in range(nq):
                ps = pp.tile([128, 512], F32, tag="tp")
                nc.tensor.transpose(ps[:, :nl], w_sb[:, q::nq], ident)
                nc.scalar.copy(wT_blk[:, q, :], ps[:, :nl])
            for grp in range(nl // IB):
                H_t = hp.tile([128, IB, nq, F], BF16, tag="H")
                for ii in range(IB):
                    i = grp * IB + ii
                    for q in range(nq):
                        if q < nq - ACT_Q:
                            nc.vector.tensor_scalar_mul(
                                H_t[:, ii, q, :], Vp_blk[:, q, :],
                                wT_blk[:, q, i : i + 1],
                            )
                        else:
                            nc.scalar.activation(
                                H_t[:, ii, q, :], Vp_blk[:, q, :],
                                ident_fn, scale=wT_blk[:, q, i : i + 1],
                            )
                nc.gpsimd.dma_start(
                    H_d[grp * IB : (grp + 1) * IB].rearrange(
                        "i (p q) f -> p i q f", q=nq
                    ),
                    H_t,
                )

    nc.compile()
    return nc


_NC_CACHE = {}


def _get_nc():
    if "nc" not in _NC_CACHE:
        _NC_CACHE["nc"] = build_kernel()
    return _NC_CACHE["nc"]


def make_in_maps(V, E, time_enc, W_weight, W_bias, a):
    V = np.asarray(V, dtype=np.float32)
    E = np.asarray(E, dtype=np.float32)
    te_bf = np.asarray(time_enc, dtype=np.float32).astype(NP_BF16)
    W_weight = np.asarray(W_weight, dtype=np.float32)
    W_bias = np.asarray(W_bias, dtype=np.float32).reshape(F, 1)
    a = np.asarray(a, dtype=np.float32)
    atb = np.ascontiguousarray(a[2 * F : 3 * F, :]).astype(NP_BF16)
    ajr = np.ascontiguousarray(
        np.broadcast_to(a[F : 2 * F, :].astype(NP_BF16), (F, 128))
    )
    in_maps = []
    for c in range(M):
        sl = slice(c * NL, (c + 1) * NL)
        in_maps.append(
            {
                "te": np.ascontiguousarray(te_bf[sl]),
                "E": np.ascontiguousarray(E[sl]),
                "V": V,
                "Vloc": np.ascontiguousarray(V[sl]),
                "W": W_weight,
                "b": W_bias,
                "a": a,
                "atb": atb,
                "ajr": ajr,
            }
        )
    return in_maps


def kernel(V, E, time_enc, W_weight, W_bias, a):
    from concourse.bass_utils import run_bass_kernel_spmd

    nc = _get_nc()
    in_maps = make_in_maps(V, E, time_enc, W_weight, W_bias, a)
    res = run_bass_kernel_spmd(nc, in_maps, core_ids=list(range(M)))
    return np.concatenate(
        [np.asarray(res.results[c]["H"]).astype(np.float32) for c in range(M)],
        axis=0,
    )

